# revision 2
# baseline (speedup 1.0000x reference)
"""Sparse ConvTranspose3d (gather + GEMM + scatter-add) on 8 TRN2 NeuronCores.

Sharding: active voxels (N dim) sorted spatially, split across 8 cores by the
output-row range their contributions land in; each core GEMMs its point shard
against all 27 kernel offsets and scatter-adds rows into its own (halo-padded)
output slab via the Ant dma_scatter_add instruction; host sums halo overlaps.

The 27 per-offset scatters are round-robined over 4 "colored" copies of the
output slab (4 separate DRAM tensors, 4 SWDGE queues). Colors break the
WAW serialization between consecutive scatter instructions (the dependency
tracker only serializes same-tensor writes) and make concurrent RMW adds
race-free by construction: rows colliding across offsets of different colors
accumulate in different tensors and are summed on the host merge; same-color
instructions remain serialized by the tracker.

Bias is folded into the GEMM via 27 extra contraction rows (one-hot per-offset
"first contribution of this output row" masks); empty output rows get bias via
a windowed scatter from a host-built bias table.
"""
import numpy as np

import concourse.bass as bass
import concourse.bacc as bacc
import concourse.tile as tile
import concourse.mybir as mybir
from concourse.bass_utils import run_bass_kernel_spmd

N_CORES = 8
KV = 27
CIN = 64
COUT = 64
N_OUT = 1620000
SLAB = N_OUT // N_CORES          # 202500
MARGIN = 8192                    # halo rows on each side of a core's slab
SC_PTS = 896                     # points per scatter instruction (7 chunks)
CPS = SC_PTS // 128              # chunks per superchunk
KAUG = CIN + KV                  # 91 contraction rows (feats + firstmask)
WCOLS = KV * COUT                # 1728
WIN = 25344
ECAP = 2048
NCOLOR = 4                       # colored output slabs / SWDGE queues

_prog_cache = {}


def _build_program(NSC, bases, ebases, work_rows):
    """Build the SPMD Bass program (same for all cores)."""
    NPTS = NSC * SC_PTS
    IDXW = SC_PTS // 16
    nc = bacc.Bacc("TRN2", target_bir_lowering=False, debug=False,
                   enable_asserts=False, num_devices=N_CORES,
                   dynamic_dma_scratch_size=65536, num_swdge_queues=NCOLOR)
    ft = nc.dram_tensor("ft", [KAUG, NPTS], mybir.dt.float32, kind="ExternalInput")
    wt = nc.dram_tensor("wt", [KAUG, WCOLS], mybir.dt.float32, kind="ExternalInput")
    idx = nc.dram_tensor("idx", [NSC, 128, KV * IDXW], mybir.dt.int16,
                         kind="ExternalInput")
    ne = max(1, len(ebases))
    esrc = nc.dram_tensor("esrc", [ne, ECAP, COUT], mybir.dt.float32,
                          kind="ExternalInput")
    eidx = nc.dram_tensor("eidx", [ne, 128, ECAP // 16], mybir.dt.int16,
                          kind="ExternalInput")
    works = [nc.dram_tensor(f"work{c}", [work_rows, COUT], mybir.dt.float32,
                            kind="ExternalOutput") for c in range(NCOLOR)]

    with tile.TileContext(nc) as tc:
        with (
            tc.tile_pool(name="const", bufs=1) as cpool,
            tc.tile_pool(name="cbuf", bufs=2) as cbpool,
            tc.tile_pool(name="ipool", bufs=3) as ipool,
            tc.tile_pool(name="psum", bufs=2, space="PSUM") as ppool,
        ):
            ft_t = cpool.tile([KAUG, NPTS], mybir.dt.float32)
            wt_t = cpool.tile([KAUG, WCOLS], mybir.dt.float32)
            nc.sync.dma_start(out=ft_t[:], in_=ft[:])
            nc.sync.dma_start(out=wt_t[:], in_=wt[:])

            for sc in range(NSC):
                c_t = cbpool.tile([128, KV, CPS, COUT], mybir.dt.float32)
                i_t = ipool.tile([128, KV * IDXW], mybir.dt.int16)
                nc.sync.dma_start(out=i_t[:], in_=idx[sc])
                for ci in range(CPS):
                    ch = sc * CPS + ci
                    ps = ppool.tile([128, WCOLS], mybir.dt.float32, space="PSUM")
                    for mm in range(4):
                        n0 = mm * 512
                        n1 = min(n0 + 512, WCOLS)
                        nc.tensor.matmul(
                            out=ps[:, n0:n1],
                            lhsT=ft_t[:, ch * 128:(ch + 1) * 128],
                            rhs=wt_t[:, n0:n1],
                            start=True, stop=True)
                    nc.vector.tensor_copy(
                        out=c_t[:, :, ci, :],
                        in_=ps[:].rearrange("p (k e) -> p k e", e=COUT))
                for k in range(KV):
                    base = bases[sc * KV + k]
                    col = k % NCOLOR
                    nc.gpsimd.dma_scatter_add(
                        works[col][base:base + 32768, :],
                        c_t[:, k, :, :],
                        i_t[:, k * IDXW:(k + 1) * IDXW],
                        SC_PTS, SC_PTS, COUT,
                        queue_num=col)

            # bias for empty output rows
            for w, base in enumerate(ebases):
                e_t = ipool.tile([128, ECAP // 128, COUT], mybir.dt.float32)
                nc.sync.dma_start(
                    out=e_t[:],
                    in_=esrc[w].rearrange("(c p) e -> p c e", p=128))
                ei_t = ipool.tile([128, ECAP // 16], mybir.dt.int16)
                nc.sync.dma_start(out=ei_t[:], in_=eidx[w])
                col = w % NCOLOR
                nc.gpsimd.dma_scatter_add(
                    works[col][base:base + 32768, :],
                    e_t[:], ei_t[:], ECAP, ECAP, COUT,
                    queue_num=col)
    nc.compile()
    return nc


def _wrap16(vals, cap):
    """int16 idx layout: token i at [i%16, i//16], replicated to 128 partitions."""
    a = np.zeros(cap, np.int16)
    a[:len(vals)] = vals
    blk = a.reshape(cap // 16, 16).T            # [16, cap/16]
    return np.tile(blk, (8, 1))                 # [128, cap/16]


def kernel(feats, weight, bias, out_index, n_out):
    feats = np.asarray(feats, np.float32)
    weight = np.asarray(weight, np.float32)
    bias = np.asarray(bias, np.float32)
    oi = np.asarray(out_index, np.int32)

    # ---- sort points spatially; merge duplicate-coordinate points ----
    order = np.argsort(oi[0], kind="stable")
    b0 = oi[0][order]
    dup = np.zeros(len(order), bool)
    dup[1:] = b0[1:] == b0[:-1]
    heads = np.where(~dup, np.arange(len(order)), 0)
    np.maximum.accumulate(heads, out=heads)
    f_s = feats[order].copy()
    if dup.any():
        np.add.at(f_s, heads[dup], f_s[np.flatnonzero(dup)])
    keep = ~dup
    f_s = f_s[keep]
    oi_s = oi[:, order[keep]]                   # [27, M] sorted, deduped
    M = oi_s.shape[1]

    # ---- first-contribution mask (bias exactly once per non-empty row) ----
    flat = oi_s.reshape(-1)
    uniq, first = np.unique(flat, return_index=True)
    fm = np.zeros(KV * M, np.float32)
    fm[first] = 1.0
    fm = fm.reshape(KV, M)
    occupied = np.zeros(n_out, bool)
    occupied[uniq] = True
    empties = np.flatnonzero(~occupied)

    # ---- assign points to cores by the slab their center-offset row hits ----
    core_of = np.minimum(oi_s[KV // 2] // SLAB, N_CORES - 1)
    counts = np.bincount(core_of, minlength=N_CORES)
    NSC = int(np.ceil(counts.max() / SC_PTS))
    NPTS = NSC * SC_PTS
    work_rows = 2 * MARGIN + SLAB + 32768      # slack so every window slice fits

    pts = [np.flatnonzero(core_of == c) for c in range(N_CORES)]

    # per-(sc,k) window bases: min over cores of the run's physical rows
    bases = np.zeros(NSC * KV, np.int64)
    phys = [None] * N_CORES
    for c in range(N_CORES):
        p = pts[c]
        ph = oi_s[:, p] - c * SLAB + MARGIN    # [27, cnt] physical slab rows
        phys[c] = ph
    for sc in range(NSC):
        lo, hi = sc * SC_PTS, (sc + 1) * SC_PTS
        for k in range(KV):
            mn, mx = work_rows, 0
            for c in range(N_CORES):
                seg = phys[c][k, lo:min(hi, len(pts[c]))]
                if len(seg):
                    mn = min(mn, seg.min())
                    mx = max(mx, seg.max())
            if mx == 0 and mn == work_rows:
                mn, mx = 0, 0
            assert mx - mn < 32768, f"window span {mx-mn} at sc={sc} k={k}"
            bases[sc * KV + k] = mn

    # ---- empties: windowed bias writes, chunked into ECAP-row instructions ----
    n_ewin = (2 * MARGIN + SLAB) // WIN + 1
    e_by = [[[] for _ in range(n_ewin)] for _ in range(N_CORES)]
    ec = np.minimum(empties // SLAB, N_CORES - 1)
    for c in range(N_CORES):
        ephys = empties[ec == c] - c * SLAB + MARGIN
        ws = ephys // WIN
        for w in range(n_ewin):
            e_by[c][w] = ephys[ws == w] - w * WIN
    ebases = []
    e_slices = []   # (w, chunk index)
    for w in range(n_ewin):
        need = max(len(e_by[c][w]) for c in range(N_CORES))
        for i in range(int(np.ceil(need / ECAP))):
            ebases.append(w * WIN)
            e_slices.append((w, i))

    key = (NSC, tuple(bases), tuple(ebases), work_rows)
    if key not in _prog_cache:
        _prog_cache[key] = _build_program(NSC, bases, ebases, work_rows)
    nc = _prog_cache[key]

    # ---- per-core input arrays ----
    IDXW = SC_PTS // 16
    wt_aug = np.zeros((KAUG, WCOLS), np.float32)
    for k in range(KV):
        wt_aug[:CIN, k * COUT:(k + 1) * COUT] = weight[k].T
        wt_aug[CIN + k, k * COUT:(k + 1) * COUT] = bias
    in_maps = []
    for c in range(N_CORES):
        p = pts[c]
        cnt = len(p)
        ft_aug = np.zeros((KAUG, NPTS), np.float32)
        ft_aug[:CIN, :cnt] = f_s[p].T
        ft_aug[CIN:, :cnt] = fm[:, p]
        idx_np = np.zeros((NSC, 128, KV * IDXW), np.int16)
        for sc in range(NSC):
            lo = sc * SC_PTS
            hi = max(lo, min(lo + SC_PTS, cnt))
            for k in range(KV):
                base = bases[sc * KV + k]
                if hi > lo:
                    offs = phys[c][k, lo:hi] - base
                    mxo = offs.max()
                    pad = mxo + 1 if mxo + 1 < 32768 else offs.min() - 1
                else:
                    offs = np.zeros(0, np.int64)
                    pad = 0
                full = np.full(SC_PTS, pad, np.int64)
                full[:hi - lo] = offs
                idx_np[sc, :, k * IDXW:(k + 1) * IDXW] = _wrap16(
                    full.astype(np.int16), SC_PTS)
        ne = max(1, len(ebases))
        esrc_np = np.zeros((ne, ECAP, COUT), np.float32)
        eidx_np = np.zeros((ne, 128, ECAP // 16), np.int16)
        for j, (w, i) in enumerate(e_slices):
            offs = np.asarray(e_by[c][w][i * ECAP:(i + 1) * ECAP], np.int64)
            esrc_np[j, :len(offs)] = bias
            pad = (offs.max() + 1) if len(offs) else 0
            if pad >= 32768:
                pad = (offs.min() - 1) if len(offs) else 0
            full = np.full(ECAP, pad, np.int64)
            full[:len(offs)] = offs
            eidx_np[j] = _wrap16(full.astype(np.int16), ECAP)
        in_maps.append({"ft": ft_aug, "wt": wt_aug, "idx": idx_np,
                        "esrc": esrc_np, "eidx": eidx_np})

    res = run_bass_kernel_spmd(nc, in_maps, list(range(N_CORES)))

    # ---- merge halo-overlapped, color-split slabs ----
    out = np.zeros((n_out, COUT), np.float32)
    for c in range(N_CORES):
        lo = c * SLAB - MARGIN
        g0, g1 = max(0, lo), min(int(n_out), (c + 1) * SLAB + MARGIN)
        for col in range(NCOLOR):
            sl = res.results[c][f"work{col}"]
            out[g0:g1] += sl[g0 - lo:g1 - lo]
    return out


# revision 7
# speedup vs baseline: 3.1712x; 3.1712x over previous
"""Sparse ConvTranspose3d (gather + GEMM + scatter-add) on 8 TRN2 NeuronCores.

Sharding: active voxels (N dim) sorted spatially, split across 8 cores by the
output-row range their contributions land in; each core GEMMs its point shard
against all 27 kernel offsets and scatter-adds rows into its own (halo-padded)
output slab via the Ant dma_scatter_add instruction; host sums halo overlaps.

The 27 per-offset scatters are round-robined over 4 "colored" copies of the
output slab (4 separate DRAM tensors, 4 SWDGE queues). Colors break the
WAW serialization between consecutive scatter instructions (the dependency
tracker only serializes same-tensor writes) and make concurrent RMW adds
race-free by construction: rows colliding across offsets of different colors
accumulate in different tensors and are summed on the host merge; same-color
instructions remain serialized by the tracker.

Bias is folded into the GEMM via 27 extra contraction rows (one-hot per-offset
"first contribution of this output row" masks); empty output rows get bias via
a windowed scatter from a host-built bias table.
"""
import numpy as np

import concourse.bass as bass
import concourse.bacc as bacc
import concourse.tile as tile
import concourse.mybir as mybir
from concourse.bass_utils import run_bass_kernel_spmd

N_CORES = 8
KV = 27
CIN = 64
COUT = 64
N_OUT = 1620000
SLAB = N_OUT // N_CORES          # 202500
MARGIN = 8192                    # halo rows on each side of a core's slab
SC_PTS = 896                     # points per scatter instruction (7 chunks)
CPS = SC_PTS // 128              # chunks per superchunk
KAUG = CIN + KV                  # 91 contraction rows (feats + firstmask)
WCOLS = KV * COUT                # 1728
WIN = 25344
ECAP = 2048
NCOLOR = 4                       # colored output slabs / SWDGE queues

_prog_cache = {}


def _build_program(NSC, bases, work_rows):
    """Build the SPMD Bass program (same for all cores)."""
    NPTS = NSC * SC_PTS
    IDXW = SC_PTS // 16
    nc = bacc.Bacc("TRN2", target_bir_lowering=False, debug=False,
                   enable_asserts=False, num_devices=N_CORES,
                   dynamic_dma_scratch_size=65536, num_swdge_queues=NCOLOR)
    ft = nc.dram_tensor("ft", [KAUG, NPTS], mybir.dt.float32, kind="ExternalInput")
    wt = nc.dram_tensor("wt", [KAUG, WCOLS], mybir.dt.float32, kind="ExternalInput")
    idx = nc.dram_tensor("idx", [NSC, 128, KV * IDXW], mybir.dt.int16,
                         kind="ExternalInput")
    works = [nc.dram_tensor(f"work{c}", [work_rows, COUT], mybir.dt.float32,
                            kind="ExternalOutput") for c in range(NCOLOR)]

    with tile.TileContext(nc) as tc:
        with (
            tc.tile_pool(name="const", bufs=1) as cpool,
            tc.tile_pool(name="cbuf", bufs=2) as cbpool,
            tc.tile_pool(name="ipool", bufs=3) as ipool,
            tc.tile_pool(name="psum", bufs=2, space="PSUM") as ppool,
        ):
            ft_t = cpool.tile([KAUG, NPTS], mybir.dt.float32)
            wt_t = cpool.tile([KAUG, WCOLS], mybir.dt.float32)
            nc.sync.dma_start(out=ft_t[:], in_=ft[:])
            nc.sync.dma_start(out=wt_t[:], in_=wt[:])

            for sc in range(NSC):
                c_t = cbpool.tile([128, KV, CPS, COUT], mybir.dt.float32)
                i_t = ipool.tile([128, KV * IDXW], mybir.dt.int16)
                nc.sync.dma_start(out=i_t[:], in_=idx[sc])
                for ci in range(CPS):
                    ch = sc * CPS + ci
                    ps = ppool.tile([128, WCOLS], mybir.dt.float32, space="PSUM")
                    for mm in range(4):
                        n0 = mm * 512
                        n1 = min(n0 + 512, WCOLS)
                        nc.tensor.matmul(
                            out=ps[:, n0:n1],
                            lhsT=ft_t[:, ch * 128:(ch + 1) * 128],
                            rhs=wt_t[:, n0:n1],
                            start=True, stop=True)
                    nc.vector.tensor_copy(
                        out=c_t[:, :, ci, :],
                        in_=ps[:].rearrange("p (k e) -> p k e", e=COUT))
                for k in range(KV):
                    base = bases[sc * KV + k]
                    col = k % NCOLOR
                    nc.gpsimd.dma_scatter_add(
                        works[col][base:base + 32768, :],
                        c_t[:, k, :, :],
                        i_t[:, k * IDXW:(k + 1) * IDXW],
                        SC_PTS, SC_PTS, COUT,
                        queue_num=col)
    nc.compile()
    return nc


def _wrap16(vals, cap):
    """int16 idx layout: token i at [i%16, i//16], replicated to 128 partitions."""
    a = np.zeros(cap, np.int16)
    a[:len(vals)] = vals
    blk = a.reshape(cap // 16, 16).T            # [16, cap/16]
    return np.tile(blk, (8, 1))                 # [128, cap/16]


def kernel(feats, weight, bias, out_index, n_out):
    feats = np.asarray(feats, np.float32)
    weight = np.asarray(weight, np.float32)
    bias = np.asarray(bias, np.float32)
    oi = np.asarray(out_index, np.int32)

    # ---- sort points spatially; merge duplicate-coordinate points ----
    order = np.argsort(oi[0], kind="stable")
    b0 = oi[0][order]
    dup = np.zeros(len(order), bool)
    dup[1:] = b0[1:] == b0[:-1]
    heads = np.where(~dup, np.arange(len(order)), 0)
    np.maximum.accumulate(heads, out=heads)
    f_s = feats[order].copy()
    if dup.any():
        np.add.at(f_s, heads[dup], f_s[np.flatnonzero(dup)])
    keep = ~dup
    f_s = f_s[keep]
    oi_s = oi[:, order[keep]]                   # [27, M] sorted, deduped
    M = oi_s.shape[1]

    # ---- first-contribution mask (bias exactly once per non-empty row) ----
    flat = oi_s.reshape(-1)
    uniq, first = np.unique(flat, return_index=True)
    fm = np.zeros(KV * M, np.float32)
    fm[first] = 1.0
    fm = fm.reshape(KV, M)
    occupied = np.zeros(n_out, bool)
    occupied[uniq] = True
    empties = np.flatnonzero(~occupied)

    # ---- assign points to cores by the slab their center-offset row hits ----
    core_of = np.minimum(oi_s[KV // 2] // SLAB, N_CORES - 1)
    counts = np.bincount(core_of, minlength=N_CORES)
    NSC = int(np.ceil(counts.max() / SC_PTS))
    NPTS = NSC * SC_PTS
    work_rows = 2 * MARGIN + SLAB + 32768      # slack so every window slice fits

    pts = [np.flatnonzero(core_of == c) for c in range(N_CORES)]

    # per-(sc,k) window bases: min over cores of the run's physical rows
    bases = np.zeros(NSC * KV, np.int64)
    phys = [None] * N_CORES
    for c in range(N_CORES):
        p = pts[c]
        ph = oi_s[:, p] - c * SLAB + MARGIN    # [27, cnt] physical slab rows
        phys[c] = ph
    for sc in range(NSC):
        lo, hi = sc * SC_PTS, (sc + 1) * SC_PTS
        for k in range(KV):
            mn, mx = work_rows, 0
            for c in range(N_CORES):
                seg = phys[c][k, lo:min(hi, len(pts[c]))]
                if len(seg):
                    mn = min(mn, seg.min())
                    mx = max(mx, seg.max())
            if mx == 0 and mn == work_rows:
                mn, mx = 0, 0
            assert mx - mn < 32768, f"window span {mx-mn} at sc={sc} k={k}"
            bases[sc * KV + k] = mn

    key = (NSC, tuple(bases), work_rows)
    if key not in _prog_cache:
        _prog_cache[key] = _build_program(NSC, bases, work_rows)
    nc = _prog_cache[key]

    # ---- per-core input arrays ----
    IDXW = SC_PTS // 16
    wt_aug = np.zeros((KAUG, WCOLS), np.float32)
    for k in range(KV):
        wt_aug[:CIN, k * COUT:(k + 1) * COUT] = weight[k].T
        wt_aug[CIN + k, k * COUT:(k + 1) * COUT] = bias
    in_maps = []
    for c in range(N_CORES):
        p = pts[c]
        cnt = len(p)
        ft_aug = np.zeros((KAUG, NPTS), np.float32)
        ft_aug[:CIN, :cnt] = f_s[p].T
        ft_aug[CIN:, :cnt] = fm[:, p]
        idx_np = np.zeros((NSC, 128, KV * IDXW), np.int16)
        for sc in range(NSC):
            lo = sc * SC_PTS
            hi = max(lo, min(lo + SC_PTS, cnt))
            for k in range(KV):
                base = bases[sc * KV + k]
                if hi > lo:
                    offs = phys[c][k, lo:hi] - base
                    mxo = offs.max()
                    pad = mxo + 1 if mxo + 1 < 32768 else offs.min() - 1
                else:
                    offs = np.zeros(0, np.int64)
                    pad = 0
                full = np.full(SC_PTS, pad, np.int64)
                full[:hi - lo] = offs
                idx_np[sc, :, k * IDXW:(k + 1) * IDXW] = _wrap16(
                    full.astype(np.int16), SC_PTS)
        in_maps.append({"ft": ft_aug, "wt": wt_aug, "idx": idx_np})

    res = run_bass_kernel_spmd(nc, in_maps, list(range(N_CORES)))

    # ---- merge halo-overlapped, color-split slabs ----
    out = np.zeros((n_out, COUT), np.float32)
    for c in range(N_CORES):
        lo = c * SLAB - MARGIN
        g0, g1 = max(0, lo), min(int(n_out), (c + 1) * SLAB + MARGIN)
        for col in range(NCOLOR):
            sl = res.results[c][f"work{col}"]
            out[g0:g1] += sl[g0 - lo:g1 - lo]
    out[empties] = bias            # bias-only rows: place the input vector
    return out
